# revision 31
# baseline (speedup 1.0000x reference)
"""GAT (2-layer graph attention) Trainium2 Bass kernel, 8-core SPMD.

Sharding: data-parallel over batch (2) x row-blocks (4) -> 8 cores.
Core c handles batch b=c//4, output rows R=[512*(c%4), 512*(c%4+1)).

Key algebra: with z = s_src[i]+s_tgt[j], the GAT edge weight
exp(leaky_relu(z, 0.2)) = max(exp(z), exp(0.2 z)).  Softmax rows are
invariant to a per-row scale, so dividing row i by exp(s_src[i]) gives
unnormalized weights F[j,i] = adjT[j,i] * D[j] * max(W[j], g[i]) with
  W[j] = exp(0.8 s_tgt[j]),  D[j] = exp(0.2 s_tgt[j]),  g[i] = exp(-0.8 s_src[i])
-- no per-element transcendentals.  Per (head, j-tile) the n x n work is
one 4x-mode tensor_scalar (DVE) + one 2x tensor_tensor mask-multiply
(split DVE/Pool), feeding TensorE numerator matmuls in bf16 with a
ones-column so the softmax denominator falls out as matmul row 64.

Host-side shard prep passes x^T / adj^T / W^T slices so every transpose
is a plain strided DMA load (no PE transposes; adjacency is cast to
bf16 host-side -- {0,1} is exact).  The layer-1 -> layer-2 row exchange
is an AllGather of each core's [i, 65] proj2|s_tgt2 block within its
batch group of 4 cores.
"""

import os
import sys

for _p in ("/opt/trn_rl_repo", "/root/.axon_site/_ro/trn_rl_repo"):
    if os.path.isdir(_p) and _p not in sys.path:
        sys.path.insert(0, _p)

import ml_dtypes
import numpy as np

import concourse.bass as bass
import concourse.bacc as bacc
import concourse.mybir as mybir
from concourse import tile
from concourse.bass_utils import run_bass_kernel_spmd

F32 = mybir.dt.float32
BF16 = mybir.dt.bfloat16
AF = mybir.ActivationFunctionType
ALU = mybir.AluOpType

BS, N, FIN = 2, 2048, 128
H1, F1 = 8, 64
RB = 512          # row block per core
NJT = N // 128    # 16 j-tiles
NCORES = 8
# j-tiles handled by DVE for the mask-multiply (rest go to gpsimd/Pool)
DVE_TT_JTS = (0, 3, 6, 9, 12)
DVE_TT_JTS2 = (0, 3, 6, 9, 12)


def build_nc():
    nc = bacc.Bacc("TRN2", target_bir_lowering=False, debug=False,
                   num_devices=NCORES)

    # ---- per-core DRAM I/O (host passes transposed/permuted shards) ----
    d_xT = nc.declare_dram_parameter("xT", [FIN, N], F32, isOutput=False)
    d_xrT = nc.declare_dram_parameter("xrT", [FIN, RB], F32, isOutput=False)
    d_adjT = nc.declare_dram_parameter("adjT", [N, RB], BF16, isOutput=False)
    d_w1T = nc.declare_dram_parameter("w1T", [FIN, H1 * F1], F32, isOutput=False)
    d_w1 = nc.declare_dram_parameter("w1", [H1 * F1, FIN], F32, isOutput=False)
    d_ws1T = nc.declare_dram_parameter("ws1T", [FIN, H1 * F1], F32, isOutput=False)
    d_as1 = nc.declare_dram_parameter("asrc1", [H1, F1], F32, isOutput=False)
    d_at1 = nc.declare_dram_parameter("atgt1", [H1, F1], F32, isOutput=False)
    d_b1 = nc.declare_dram_parameter("b1", [H1 * F1], F32, isOutput=False)
    d_w2 = nc.declare_dram_parameter("w2", [F1, H1 * F1], F32, isOutput=False)
    d_w2T = nc.declare_dram_parameter("w2T", [H1 * F1, F1], F32, isOutput=False)
    d_ws2T = nc.declare_dram_parameter("ws2T", [H1 * F1, F1], F32, isOutput=False)
    d_as2 = nc.declare_dram_parameter("asrc2", [1, F1], F32, isOutput=False)
    d_at2 = nc.declare_dram_parameter("atgt2", [1, F1], F32, isOutput=False)
    d_b2 = nc.declare_dram_parameter("b2", [F1], F32, isOutput=False)
    # output: transposed row-block out^T [64, 512] (host transposes back)
    d_out = nc.declare_dram_parameter("outT", [F1, RB], F32, isOutput=True)

    GROUPS = [[0, 1, 2, 3], [4, 5, 6, 7]]

    with tile.TileContext(nc) as tc:
        with (
            tc.tile_pool(name="persist", bufs=1) as P,
            tc.tile_pool(name="work", bufs=4) as WK,
            tc.tile_pool(name="qf", bufs=10) as QF,
            tc.tile_pool(name="ps", bufs=3, space="PSUM") as PS,
            tc.tile_pool(name="psnum", bufs=4, space="PSUM") as PSN,
            tc.tile_pool(name="dram", bufs=1, space="DRAM") as DR,
        ):
            # ============ loads (all plain strided DMAs) ====================
            # small weights first (scalar queue), then x chunks + adjT (sync)
            w1n = P.tile([128, 4, FIN], F32, tag="w1n")
            nc.scalar.dma_start(w1n[:], d_w1.rearrange("(k p) c -> p k c", p=128))
            a1sT = P.tile([128, H1], F32, tag="a1sT")
            nc.scalar.dma_start(a1sT[0:F1, :], d_as1.rearrange("h f -> f h"))
            nc.scalar.dma_start(a1sT[F1:2 * F1, :], d_as1.rearrange("h f -> f h"))
            a1tT = P.tile([128, H1], F32, tag="a1tT")
            nc.scalar.dma_start(a1tT[0:F1, :], d_at1.rearrange("h f -> f h"))
            nc.scalar.dma_start(a1tT[F1:2 * F1, :], d_at1.rearrange("h f -> f h"))

            xrTf = P.tile([128, RB], F32, tag="xrTf")
            nc.sync.dma_start(xrTf[:], d_xrT[:, :])
            w1Tf = P.tile([128, H1 * F1], F32, tag="w1Tf")
            nc.sync.dma_start(w1Tf[:], d_w1T[:, :])
            xT = P.tile([128, N], F32, tag="xT")
            adjT = P.tile([128, NJT, RB], BF16, tag="adjT")
            adjq = d_adjT.rearrange("(q t p) i -> q p t i", p=128, t=4)
            nc.sync.dma_start(xT[:, 0:512], d_xT[:, 0:512])
            nc.sync.dma_start(adjT[:, 0:4, :], adjq[0])
            ws1Tf = P.tile([128, H1 * F1], F32, tag="ws1Tf")
            nc.sync.dma_start(ws1Tf[:], d_ws1T[:, :])
            b1f = P.tile([128, 4], F32, tag="b1f")
            nc.sync.dma_start(b1f[:], d_b1.rearrange("(k p) -> p k", p=128))
            for q in range(1, 4):
                nc.sync.dma_start(xT[:, q * 512:(q + 1) * 512],
                                  d_xT[:, q * 512:(q + 1) * 512])
                nc.sync.dma_start(adjT[:, 4 * q:4 * q + 4, :], adjq[q])
            w2n = P.tile([F1, H1 * F1], F32, tag="w2n")
            nc.sync.dma_start(w2n[:], d_w2[:, :])
            w2Tf = P.tile([128, 4, F1], F32, tag="w2Tf")
            nc.sync.dma_start(w2Tf[:], d_w2T.rearrange("(k p) f -> p k f", p=128))
            ws2Tf = P.tile([128, 4, F1], F32, tag="ws2Tf")
            nc.sync.dma_start(ws2Tf[:], d_ws2T.rearrange("(k p) f -> p k f", p=128))
            a2p = P.tile([F1, 2], F32, tag="a2p")
            nc.scalar.dma_start(a2p[:, 0:1], d_as2.rearrange("o f -> f o"))
            nc.scalar.dma_start(a2p[:, 1:2], d_at2.rearrange("o f -> f o"))
            b2f = P.tile([F1, 1], F32, tag="b2f")
            nc.sync.dma_start(b2f[:], d_b2.ap().rearrange("(f o) -> f o", o=1))

            # bf16 casts (DVE)
            xrTb = P.tile([128, RB], BF16, tag="xrTb")
            nc.vector.tensor_copy(xrTb[:], xrTf[:])
            xTb = P.tile([128, N], BF16, tag="xTb")
            for q in range(4):
                nc.vector.tensor_copy(xTb[:, q * 512:(q + 1) * 512],
                                      xT[:, q * 512:(q + 1) * 512])
            w1Tb = P.tile([128, H1 * F1], BF16, tag="w1Tb")
            nc.vector.tensor_copy(w1Tb[:], w1Tf[:])
            ws1Tb = P.tile([128, H1 * F1], BF16, tag="ws1Tb")
            nc.vector.tensor_copy(ws1Tb[:], ws1Tf[:])
            w2Tb = P.tile([128, 4, F1], BF16, tag="w2Tb")
            nc.vector.tensor_copy(w2Tb[:], w2Tf[:])
            ws2Tb = P.tile([128, 4, F1], BF16, tag="ws2Tb")
            nc.vector.tensor_copy(ws2Tb[:], ws2Tf[:])
            b2sb = P.tile([F1, 1], BF16, tag="b2sb")
            nc.vector.tensor_copy(b2sb[:], b2f[:])
            ones1b = P.tile([1, 128], BF16, tag="ones1b")
            nc.vector.memset(ones1b[:], 1.0)

            # ============ small exact fp32 matmuls ==========================
            # w1tilde [c=128, 16]: col h = W1_h^T a_src1[h], col 8+h tgt
            ps_wt = PS.tile([128, 512], F32, tag="ps")
            for h in range(H1):
                kt, pr = (h * F1) // 128, (h * F1) % 128
                w1slc = w1n[pr:pr + F1, kt, :]
                nc.tensor.matmul(ps_wt[0:128, h:h + 1], w1slc,
                                 a1sT[pr:pr + F1, h:h + 1])
                nc.tensor.matmul(ps_wt[0:128, 8 + h:9 + h], w1slc,
                                 a1tT[pr:pr + F1, h:h + 1])
            w1t = P.tile([128, 16], F32, tag="w1t")
            nc.vector.tensor_copy(w1t[:], ps_wt[0:128, 0:16])

            # s_src rows for our block (block 0 = own rows) -> g1b bf16
            ps_s1r = PS.tile([128, 512], F32, tag="ps")
            nc.tensor.matmul(ps_s1r[0:16, 0:RB], w1t[:], xrTf[:])
            g1b = P.tile([16, RB], BF16, tag="g1b")
            nc.scalar.activation(g1b[:], ps_s1r[0:16, 0:RB], AF.Exp, scale=-0.8)
            # selector tiles sel_h [16, 128] (row h ones) for g broadcasts
            ones16 = P.tile([16, 128], BF16, tag="ones16")
            nc.vector.memset(ones16[:], 1.0)
            sel = P.tile([16, H1 * 128], BF16, tag="sel")
            for h in range(H1):
                nc.gpsimd.affine_select(sel[:, h * 128:(h + 1) * 128],
                                        ones16[:], [[0, 128]], ALU.is_equal,
                                        0.0, base=-h, channel_multiplier=1)
            gb_all = P.tile([128, 8, RB], BF16, tag="gb_all")
            for h in range(H1):
                ps_g = PS.tile([128, 512], F32, tag="ps")
                nc.tensor.matmul(ps_g[0:128, 0:RB],
                                 sel[:, h * 128:(h + 1) * 128], g1b[:])
                if h % 2 == 0:
                    nc.scalar.activation(gb_all[:, h, :], ps_g[0:128, 0:RB],
                                         AF.Copy)
                else:
                    nc.vector.tensor_copy(gb_all[:, h, :], ps_g[0:128, 0:RB])

            # S1T [j(128 x 16 chunks), 16] = x @ w1tilde ; Wv/Dv = exp scales
            s1T = P.tile([128, NJT * 16], F32, tag="s1T")
            Wvf = P.tile([128, NJT * 16], F32, tag="Wvf")
            Dvf = P.tile([128, NJT * 16], F32, tag="Dvf")
            for q in range(4):
                ps_s1t = PS.tile([128, 512], F32, tag="ps")
                for jc in range(4 * q, 4 * q + 4):
                    nc.tensor.matmul(ps_s1t[0:128, (jc % 4) * 16:(jc % 4) * 16 + 16],
                                     xT[:, jc * 128:(jc + 1) * 128], w1t[:])
                sl = slice(q * 64, (q + 1) * 64)
                nc.vector.tensor_copy(s1T[:, sl], ps_s1t[0:128, 0:64])
                nc.scalar.activation(Wvf[:, sl], s1T[:, sl], AF.Exp, scale=0.8)
                nc.scalar.activation(Dvf[:, sl], s1T[:, sl], AF.Exp, scale=0.2)

            skipTb = P.tile([128, 4, RB], BF16, tag="skipTb")
            # ============ proj1 (+ ones col) ================================
            p1e = P.tile([128, NJT, 8 * 66], BF16, tag="p1e")
            nc.vector.memset(
                p1e[:].rearrange("p j (h q) -> p j h q", q=66)[:, :, :, 64:65],
                1.0)
            for jt in range(NJT):
                ps_p = PS.tile([128, 512], F32, tag="ps")
                nc.tensor.matmul(ps_p[0:128, 0:512],
                                 xTb[:, jt * 128:(jt + 1) * 128], w1Tb[:])
                dst = p1e[:, jt, :].rearrange("p (h q) -> p h q", q=66)
                src = ps_p[0:128, 0:512].rearrange("p (h q) -> p h q", q=64)
                if jt % 2 == 0:
                    nc.scalar.activation(dst[:, :, 0:64], src, AF.Copy)
                else:
                    nc.vector.tensor_copy(dst[:, :, 0:64], src)

            # ============ layer-1 head loop =================================
            numb = P.tile([128, 4, RB], BF16, tag="numb")
            rdb = P.tile([128, 4, RB], BF16, tag="rdb")
            recbs = []
            for h in range(H1):
                numT = PSN.tile([65, 512], F32, tag="numT")
                for jt in range(NJT):
                    col = slice(jt * 16 + 8 + h, jt * 16 + 9 + h)
                    q = QF.tile([128, RB], BF16, tag="q")
                    nc.vector.tensor_scalar(q[:], gb_all[:, h, :], Wvf[:, col],
                                            Dvf[:, col], ALU.max, ALU.mult)
                    Ft = QF.tile([128, RB], BF16, tag="F")
                    dve_set = DVE_TT_JTS if h % 2 == 0 else DVE_TT_JTS[:-1]
                    eng = nc.vector if jt in dve_set else nc.gpsimd
                    eng.tensor_tensor(Ft[:], q[:], adjT[:, jt, :], ALU.mult)
                    nc.tensor.matmul(numT[0:65, 0:RB],
                                     p1e[:, jt, h * 66:h * 66 + 65], Ft[:],
                                     start=(jt == 0), stop=(jt == NJT - 1))
                den_h = WK.tile([1, RB], F32, tag="den")
                nc.scalar.activation(den_h[:], numT[64:65, 0:RB], AF.Copy)
                rec_h = WK.tile([1, RB], F32, tag="rec")
                nc.vector.reciprocal_approx_fast(rec_h[:], den_h[:])
                recb_h = P.tile([1, RB], BF16, tag=f"recb{h}")
                nc.scalar.activation(recb_h[:], rec_h[:], AF.Copy)
                recbs.append(recb_h)
                nc.scalar.activation(numb[(h % 2) * 64:(h % 2) * 64 + 64, h // 2, :],
                                     numT[0:64, 0:RB], AF.Copy)
                if h < 4:
                    ps_sk = PS.tile([128, 512], F32, tag="ps")
                    nc.tensor.matmul(ps_sk[0:128, 0:RB],
                                     ws1Tb[:, h * 128:(h + 1) * 128], xrTb[:])
                    nc.scalar.activation(skipTb[:, h, :], ps_sk[0:128, 0:RB],
                                         AF.Identity, bias=b1f[:, h:h + 1])
                if h % 2 == 1:
                    pr = h // 2
                    ps_r = PS.tile([128, 512], F32, tag="ps")
                    nc.tensor.matmul(ps_r[0:64, 0:RB], ones1b[0:1, 0:64],
                                     recbs[2 * pr][:])
                    nc.tensor.matmul(ps_r[64:128, 0:RB], ones1b[0:1, 0:64],
                                     recbs[2 * pr + 1][:])
                    nc.scalar.activation(rdb[:, pr, :], ps_r[0:128, 0:RB], AF.Copy)

            # h_out^T = elu(num/den + (skip + b1)), kept bf16, per pair
            houtb = P.tile([128, 4, RB], BF16, tag="houtb")
            for pr in range(4):
                hpre = WK.tile([128, RB], BF16, tag="hpre")
                nc.vector.tensor_mul(hpre[:], numb[:, pr, :], rdb[:, pr, :])
                u = WK.tile([128, RB], BF16, tag="u")
                nc.gpsimd.tensor_add(u[:], hpre[:], skipTb[:, pr, :])
                m0 = WK.tile([128, RB], BF16, tag="m0")
                nc.vector.tensor_scalar(m0[:], u[:], 0.0, None, ALU.min)
                e = WK.tile([128, RB], BF16, tag="e")
                nc.scalar.activation(e[:], m0[:], AF.Exp)
                em1 = WK.tile([128, RB], BF16, tag="em1")
                nc.vector.tensor_scalar(em1[:], e[:], -1.0, None, ALU.add)
                nc.vector.tensor_tensor(houtb[:, pr, :], em1[:], u[:], ALU.max)

            # ============ layer-2 local pieces ==============================
            ps_w2 = PS.tile([128, 512], F32, tag="ps")
            for kt in range(4):
                nc.tensor.matmul(ps_w2[0:128, kt * 2:kt * 2 + 2],
                                 w2n[:, kt * 128:(kt + 1) * 128], a2p[:],
                                 start=True, stop=True)
            w2tb = P.tile([128, 8], BF16, tag="w2tb")
            nc.vector.tensor_copy(w2tb[:], ps_w2[0:128, 0:8])

            # s_src2 row -> g2 broadcast (roundtrip DMA)
            ps_s2 = PS.tile([128, 512], F32, tag="ps")
            for kt in range(4):
                nc.tensor.matmul(ps_s2[0:1, 0:RB], w2tb[:, kt * 2:kt * 2 + 1],
                                 houtb[:, kt, :], start=(kt == 0), stop=(kt == 3))
            g2row = P.tile([1, RB], BF16, tag="g2row")
            nc.scalar.activation(g2row[:], ps_s2[0:1, 0:RB], AF.Exp, scale=-0.8)
            ps_g2 = PS.tile([128, 512], F32, tag="ps")
            nc.tensor.matmul(ps_g2[0:128, 0:RB], ones1b[:], g2row[:])
            g2bc = P.tile([128, RB], BF16, tag="g2bc")
            nc.scalar.activation(g2bc[:], ps_g2[0:128, 0:RB], AF.Copy)

            # p2e slot 0 (own rows): [i, 64 proj2 cols | s_tgt2]
            # moving operand w2aug [128 f', 4kt, 65]
            w2aug = P.tile([128, 4, 66], BF16, tag="w2aug")
            for kt in range(4):
                nc.vector.tensor_copy(w2aug[:, kt, 0:64], w2Tb[:, kt, :])
                nc.vector.tensor_copy(w2aug[:, kt, 64:65],
                                      w2tb[:, kt * 2 + 1:kt * 2 + 2])
            p2own = P.tile([128, 4, F1 + 1], BF16, tag="p2own")
            p2e = P.tile([128, NJT, F1 + 1], BF16, tag="p2e")
            ps_p2 = PSN.tile([128, 512], F32, tag="numT")
            for s in range(4):
                for kt in range(4):
                    nc.tensor.matmul(
                        ps_p2[0:128, s * 65:s * 65 + 65],
                        houtb[:, kt, s * 128:(s + 1) * 128],
                        w2aug[:, kt, 0:65], start=(kt == 0), stop=(kt == 3))
            nc.scalar.activation(
                p2own[:].rearrange("p s f -> p (s f)"),
                ps_p2[0:128, 0:260], AF.Copy)

            # ============ AllGather within batch group ======================
            gin = DR.tile([4, 128, 65], BF16)
            nc.sync.dma_start(gin[:].rearrange("s p f -> p s f"), p2own[:])
            gout = DR.tile([4, 4, 128, 65], BF16)
            nc.gpsimd.collective_compute(
                "AllGather", ALU.bypass, replica_groups=GROUPS,
                ins=[gin.opt()], outs=[gout.opt()])
            for q in range(2):
                nc.sync.dma_start(
                    p2e[:, 8 * q:8 * q + 8, :],
                    gout[2 * q:2 * q + 2].rearrange("c s p f -> p (c s) f"))

            # ============ layer-2 attention =================================
            st2 = P.tile([128, NJT], F32, tag="st2")
            nc.scalar.activation(st2[:], p2e[:, :, 64], AF.Copy)
            D2v = P.tile([128, NJT], F32, tag="D2v")
            nc.scalar.activation(D2v[:], st2[:], AF.Exp, scale=0.2)
            W2vf = P.tile([128, NJT], F32, tag="W2vf")
            nc.scalar.activation(W2vf[:], st2[:], AF.Exp, scale=0.8)
            # denominator column must be 1 for the ones-trick
            nc.vector.memset(p2e[:, :, F1:F1 + 1], 1.0)

            # bridge the PE p-state ramp (3us) while the L2 TS/TT pipeline
            # fills, so the accumulation-chain matmuls below run at 2.4GHz
            ps_warm = PS.tile([128, 512], F32, tag="ps")
            for w in range(7):
                nc.tensor.matmul(ps_warm[0:65, 0:512], p2e[:, 2 * w, :],
                                 g2bc[:], start=True, stop=True)
            numT2 = PSN.tile([65, 512], F32, tag="numT")
            for jt in range(NJT):
                q2 = QF.tile([128, RB], BF16, tag="q")
                nc.vector.tensor_scalar(q2[:], g2bc[:], W2vf[:, jt:jt + 1],
                                        D2v[:, jt:jt + 1], ALU.max, ALU.mult)
                F2 = QF.tile([128, RB], BF16, tag="F")
                eng = nc.vector if jt in DVE_TT_JTS2 else nc.gpsimd
                eng.tensor_tensor(F2[:], q2[:], adjT[:, jt, :], ALU.mult)
                nc.tensor.matmul(numT2[0:65, 0:RB], p2e[:, jt, :],
                                 F2[:], start=(jt == 0), stop=(jt == NJT - 1))

            den2 = WK.tile([1, RB], F32, tag="den")
            nc.scalar.activation(den2[:], numT2[64:65, 0:RB], AF.Copy)
            rec2f = WK.tile([1, RB], F32, tag="rec")
            nc.vector.reciprocal_approx_fast(rec2f[:], den2[:])
            rec2b = P.tile([1, RB], BF16, tag="rec2b")
            nc.vector.tensor_copy(rec2b[:], rec2f[:])
            ps_r2 = PS.tile([128, 512], F32, tag="ps")
            nc.tensor.matmul(ps_r2[0:64, 0:RB], ones1b[0:1, 0:64], rec2b[:])
            rdb2 = P.tile([F1, RB], BF16, tag="rdb2")
            nc.scalar.activation(rdb2[:], ps_r2[0:64, 0:RB], AF.Copy)

            ps_sk2 = PS.tile([128, 512], F32, tag="ps")
            for kt in range(4):
                nc.tensor.matmul(ps_sk2[0:64, 0:RB], ws2Tb[:, kt, :],
                                 houtb[:, kt, :], start=(kt == 0), stop=(kt == 3))

            t2 = WK.tile([F1, RB], F32, tag="t2")
            nc.vector.tensor_mul(t2[:], numT2[0:64, 0:RB], rdb2[:])
            o2 = WK.tile([F1, RB], F32, tag="o2")
            nc.vector.scalar_tensor_tensor(
                o2[:], t2[:], b2f[:], ps_sk2[0:64, 0:RB], ALU.add, ALU.add)
            nc.sync.dma_start(d_out[:, :], o2[:])

    nc.compile()
    return nc


_NC_CACHE = None


def _get_nc():
    global _NC_CACHE
    if _NC_CACHE is None:
        _NC_CACHE = build_nc()
    return _NC_CACHE


def make_in_map(inp, c):
    """Host-side shard prep for core c: transpose/permute views of inputs."""
    x = np.asarray(inp["x"], np.float32)
    adj = np.asarray(inp["adj"], np.float32)
    b, r = c // 4, c % 4
    A = adj[b][r * RB:(r + 1) * RB, :].T
    return {
        "xT": np.ascontiguousarray(x[b].T),
        "xrT": np.ascontiguousarray(x[b][r * RB:(r + 1) * RB].T),
        "adjT": np.ascontiguousarray(A).astype(ml_dtypes.bfloat16),
        "w1T": np.ascontiguousarray(np.asarray(inp["W1"], np.float32).T),
        "w1": np.asarray(inp["W1"], np.float32),
        "ws1T": np.ascontiguousarray(np.asarray(inp["Wskip1"], np.float32).T),
        "asrc1": np.asarray(inp["a_src1"], np.float32),
        "atgt1": np.asarray(inp["a_tgt1"], np.float32),
        "b1": np.asarray(inp["b1"], np.float32),
        "w2": np.asarray(inp["W2"], np.float32),
        "w2T": np.ascontiguousarray(np.asarray(inp["W2"], np.float32).T),
        "ws2T": np.ascontiguousarray(np.asarray(inp["Wskip2"], np.float32).T),
        "asrc2": np.asarray(inp["a_src2"], np.float32),
        "atgt2": np.asarray(inp["a_tgt2"], np.float32),
        "b2": np.asarray(inp["b2"], np.float32),
    }


def kernel(x, adj, W1, a_src1, a_tgt1, Wskip1, b1, W2, a_src2, a_tgt2,
           Wskip2, b2):
    inp = {"x": x, "adj": adj, "W1": W1, "a_src1": a_src1, "a_tgt1": a_tgt1,
           "Wskip1": Wskip1, "b1": b1, "W2": W2, "a_src2": a_src2,
           "a_tgt2": a_tgt2, "Wskip2": Wskip2, "b2": b2}
    nc = _get_nc()
    in_maps = [make_in_map(inp, c) for c in range(NCORES)]
    res = run_bass_kernel_spmd(nc, in_maps, core_ids=list(range(NCORES)))
    out = np.empty((BS, N, F1), np.float32)
    for c in range(NCORES):
        b, r = c // 4, c % 4
        out[b, r * RB:(r + 1) * RB, :] = res.results[c]["outT"].T
    return out


# revision 32
# speedup vs baseline: 1.0041x; 1.0041x over previous
"""GAT (2-layer graph attention) Trainium2 Bass kernel, 8-core SPMD.

Sharding: data-parallel over batch (2) x row-blocks (4) -> 8 cores.
Core c handles batch b=c//4, output rows R=[512*(c%4), 512*(c%4+1)).

Key algebra: with z = s_src[i]+s_tgt[j], the GAT edge weight
exp(leaky_relu(z, 0.2)) = max(exp(z), exp(0.2 z)).  Softmax rows are
invariant to a per-row scale, so dividing row i by exp(s_src[i]) gives
unnormalized weights F[j,i] = adjT[j,i] * D[j] * max(W[j], g[i]) with
  W[j] = exp(0.8 s_tgt[j]),  D[j] = exp(0.2 s_tgt[j]),  g[i] = exp(-0.8 s_src[i])
-- no per-element transcendentals.  Per (head, j-tile) the n x n work is
one 4x-mode tensor_scalar (DVE) + one 2x tensor_tensor mask-multiply
(split DVE/Pool), feeding TensorE numerator matmuls in bf16 with a
ones-column so the softmax denominator falls out as matmul row 64.

Host-side shard prep passes x^T / adj^T / W^T slices so every transpose
is a plain strided DMA load (no PE transposes; adjacency is cast to
bf16 host-side -- {0,1} is exact).  The layer-1 -> layer-2 row exchange
is an AllGather of each core's [i, 65] proj2|s_tgt2 block within its
batch group of 4 cores.
"""

import os
import sys

for _p in ("/opt/trn_rl_repo", "/root/.axon_site/_ro/trn_rl_repo"):
    if os.path.isdir(_p) and _p not in sys.path:
        sys.path.insert(0, _p)

import ml_dtypes
import numpy as np

import concourse.bass as bass
import concourse.bacc as bacc
import concourse.mybir as mybir
from concourse import tile
from concourse.bass_utils import run_bass_kernel_spmd

F32 = mybir.dt.float32
BF16 = mybir.dt.bfloat16
AF = mybir.ActivationFunctionType
ALU = mybir.AluOpType

BS, N, FIN = 2, 2048, 128
H1, F1 = 8, 64
RB = 512          # row block per core
NJT = N // 128    # 16 j-tiles
NCORES = 8
# j-tiles handled by DVE for the mask-multiply (rest go to gpsimd/Pool)
DVE_TT_JTS = (0, 3, 6, 9, 12)
DVE_TT_JTS2 = (0, 3, 6, 9, 12)


def build_nc():
    nc = bacc.Bacc("TRN2", target_bir_lowering=False, debug=False,
                   num_devices=NCORES)

    # ---- per-core DRAM I/O (host passes transposed/permuted shards) ----
    d_xT = nc.declare_dram_parameter("xT", [FIN, N], F32, isOutput=False)
    d_xrT = nc.declare_dram_parameter("xrT", [FIN, RB], F32, isOutput=False)
    d_adjT = nc.declare_dram_parameter("adjT", [N, RB], BF16, isOutput=False)
    d_w1T = nc.declare_dram_parameter("w1T", [FIN, H1 * F1], F32, isOutput=False)
    d_w1 = nc.declare_dram_parameter("w1", [H1 * F1, FIN], F32, isOutput=False)
    d_ws1T = nc.declare_dram_parameter("ws1T", [FIN, H1 * F1], F32, isOutput=False)
    d_as1 = nc.declare_dram_parameter("asrc1", [H1, F1], F32, isOutput=False)
    d_at1 = nc.declare_dram_parameter("atgt1", [H1, F1], F32, isOutput=False)
    d_b1 = nc.declare_dram_parameter("b1", [H1 * F1], F32, isOutput=False)
    d_w2 = nc.declare_dram_parameter("w2", [F1, H1 * F1], F32, isOutput=False)
    d_w2T = nc.declare_dram_parameter("w2T", [H1 * F1, F1], F32, isOutput=False)
    d_ws2T = nc.declare_dram_parameter("ws2T", [H1 * F1, F1], F32, isOutput=False)
    d_as2 = nc.declare_dram_parameter("asrc2", [1, F1], F32, isOutput=False)
    d_at2 = nc.declare_dram_parameter("atgt2", [1, F1], F32, isOutput=False)
    d_b2 = nc.declare_dram_parameter("b2", [F1], F32, isOutput=False)
    # output: transposed row-block out^T [64, 512] (host transposes back)
    d_out = nc.declare_dram_parameter("outT", [F1, RB], F32, isOutput=True)

    GROUPS = [[0, 1, 2, 3], [4, 5, 6, 7]]

    with tile.TileContext(nc) as tc:
        with (
            tc.tile_pool(name="persist", bufs=1) as P,
            tc.tile_pool(name="work", bufs=4) as WK,
            tc.tile_pool(name="qf", bufs=14) as QF,
            tc.tile_pool(name="ps", bufs=3, space="PSUM") as PS,
            tc.tile_pool(name="psnum", bufs=4, space="PSUM") as PSN,
            tc.tile_pool(name="dram", bufs=1, space="DRAM") as DR,
        ):
            # ============ loads (all plain strided DMAs) ====================
            # small weights first (scalar queue), then x chunks + adjT (sync)
            w1n = P.tile([128, 4, FIN], F32, tag="w1n")
            nc.scalar.dma_start(w1n[:], d_w1.rearrange("(k p) c -> p k c", p=128))
            a1sT = P.tile([128, H1], F32, tag="a1sT")
            nc.scalar.dma_start(a1sT[0:F1, :], d_as1.rearrange("h f -> f h"))
            nc.scalar.dma_start(a1sT[F1:2 * F1, :], d_as1.rearrange("h f -> f h"))
            a1tT = P.tile([128, H1], F32, tag="a1tT")
            nc.scalar.dma_start(a1tT[0:F1, :], d_at1.rearrange("h f -> f h"))
            nc.scalar.dma_start(a1tT[F1:2 * F1, :], d_at1.rearrange("h f -> f h"))

            xrTf = P.tile([128, RB], F32, tag="xrTf")
            nc.sync.dma_start(xrTf[:], d_xrT[:, :])
            w1Tf = P.tile([128, H1 * F1], F32, tag="w1Tf")
            nc.sync.dma_start(w1Tf[:], d_w1T[:, :])
            xT = P.tile([128, N], F32, tag="xT")
            adjT = P.tile([128, NJT, RB], BF16, tag="adjT")
            adjq = d_adjT.rearrange("(q t p) i -> q p t i", p=128, t=4)
            nc.sync.dma_start(xT[:, 0:512], d_xT[:, 0:512])
            nc.sync.dma_start(adjT[:, 0:4, :], adjq[0])
            ws1Tf = P.tile([128, H1 * F1], F32, tag="ws1Tf")
            nc.sync.dma_start(ws1Tf[:], d_ws1T[:, :])
            b1f = P.tile([128, 4], F32, tag="b1f")
            nc.sync.dma_start(b1f[:], d_b1.rearrange("(k p) -> p k", p=128))
            for q in range(1, 4):
                nc.sync.dma_start(xT[:, q * 512:(q + 1) * 512],
                                  d_xT[:, q * 512:(q + 1) * 512])
                nc.sync.dma_start(adjT[:, 4 * q:4 * q + 4, :], adjq[q])
            w2n = P.tile([F1, H1 * F1], F32, tag="w2n")
            nc.sync.dma_start(w2n[:], d_w2[:, :])
            w2Tf = P.tile([128, 4, F1], F32, tag="w2Tf")
            nc.sync.dma_start(w2Tf[:], d_w2T.rearrange("(k p) f -> p k f", p=128))
            ws2Tf = P.tile([128, 4, F1], F32, tag="ws2Tf")
            nc.sync.dma_start(ws2Tf[:], d_ws2T.rearrange("(k p) f -> p k f", p=128))
            a2p = P.tile([F1, 2], F32, tag="a2p")
            nc.scalar.dma_start(a2p[:, 0:1], d_as2.rearrange("o f -> f o"))
            nc.scalar.dma_start(a2p[:, 1:2], d_at2.rearrange("o f -> f o"))
            b2f = P.tile([F1, 1], F32, tag="b2f")
            nc.sync.dma_start(b2f[:], d_b2.ap().rearrange("(f o) -> f o", o=1))

            # bf16 casts (DVE)
            xrTb = P.tile([128, RB], BF16, tag="xrTb")
            nc.vector.tensor_copy(xrTb[:], xrTf[:])
            xTb = P.tile([128, N], BF16, tag="xTb")
            for q in range(4):
                nc.vector.tensor_copy(xTb[:, q * 512:(q + 1) * 512],
                                      xT[:, q * 512:(q + 1) * 512])
            w1Tb = P.tile([128, H1 * F1], BF16, tag="w1Tb")
            nc.vector.tensor_copy(w1Tb[:], w1Tf[:])
            ws1Tb = P.tile([128, H1 * F1], BF16, tag="ws1Tb")
            nc.vector.tensor_copy(ws1Tb[:], ws1Tf[:])
            w2Tb = P.tile([128, 4, F1], BF16, tag="w2Tb")
            nc.vector.tensor_copy(w2Tb[:], w2Tf[:])
            ws2Tb = P.tile([128, 4, F1], BF16, tag="ws2Tb")
            nc.vector.tensor_copy(ws2Tb[:], ws2Tf[:])
            b2sb = P.tile([F1, 1], BF16, tag="b2sb")
            nc.vector.tensor_copy(b2sb[:], b2f[:])
            ones1b = P.tile([1, 128], BF16, tag="ones1b")
            nc.vector.memset(ones1b[:], 1.0)

            # ============ small exact fp32 matmuls ==========================
            # w1tilde [c=128, 16]: col h = W1_h^T a_src1[h], col 8+h tgt
            ps_wt = PS.tile([128, 512], F32, tag="ps")
            for h in range(H1):
                kt, pr = (h * F1) // 128, (h * F1) % 128
                w1slc = w1n[pr:pr + F1, kt, :]
                nc.tensor.matmul(ps_wt[0:128, h:h + 1], w1slc,
                                 a1sT[pr:pr + F1, h:h + 1])
                nc.tensor.matmul(ps_wt[0:128, 8 + h:9 + h], w1slc,
                                 a1tT[pr:pr + F1, h:h + 1])
            w1t = P.tile([128, 16], F32, tag="w1t")
            nc.vector.tensor_copy(w1t[:], ps_wt[0:128, 0:16])

            # s_src rows for our block (block 0 = own rows) -> g1b bf16
            ps_s1r = PS.tile([128, 512], F32, tag="ps")
            nc.tensor.matmul(ps_s1r[0:16, 0:RB], w1t[:], xrTf[:])
            g1b = P.tile([16, RB], BF16, tag="g1b")
            nc.scalar.activation(g1b[:], ps_s1r[0:16, 0:RB], AF.Exp, scale=-0.8)
            # selector tiles sel_h [16, 128] (row h ones) for g broadcasts
            ones16 = P.tile([16, 128], BF16, tag="ones16")
            nc.vector.memset(ones16[:], 1.0)
            sel = P.tile([16, H1 * 128], BF16, tag="sel")
            for h in range(H1):
                nc.gpsimd.affine_select(sel[:, h * 128:(h + 1) * 128],
                                        ones16[:], [[0, 128]], ALU.is_equal,
                                        0.0, base=-h, channel_multiplier=1)
            gb_all = P.tile([128, 8, RB], BF16, tag="gb_all")
            for h in range(H1):
                ps_g = PS.tile([128, 512], F32, tag="ps")
                nc.tensor.matmul(ps_g[0:128, 0:RB],
                                 sel[:, h * 128:(h + 1) * 128], g1b[:])
                if h % 2 == 0:
                    nc.scalar.activation(gb_all[:, h, :], ps_g[0:128, 0:RB],
                                         AF.Copy)
                else:
                    nc.vector.tensor_copy(gb_all[:, h, :], ps_g[0:128, 0:RB])

            # S1T [j(128 x 16 chunks), 16] = x @ w1tilde ; Wv/Dv = exp scales
            s1T = P.tile([128, NJT * 16], F32, tag="s1T")
            Wvf = P.tile([128, NJT * 16], F32, tag="Wvf")
            Dvf = P.tile([128, NJT * 16], F32, tag="Dvf")
            for q in range(4):
                ps_s1t = PS.tile([128, 512], F32, tag="ps")
                for jc in range(4 * q, 4 * q + 4):
                    nc.tensor.matmul(ps_s1t[0:128, (jc % 4) * 16:(jc % 4) * 16 + 16],
                                     xT[:, jc * 128:(jc + 1) * 128], w1t[:])
                sl = slice(q * 64, (q + 1) * 64)
                nc.vector.tensor_copy(s1T[:, sl], ps_s1t[0:128, 0:64])
                nc.scalar.activation(Wvf[:, sl], s1T[:, sl], AF.Exp, scale=0.8)
                nc.scalar.activation(Dvf[:, sl], s1T[:, sl], AF.Exp, scale=0.2)

            skipTb = P.tile([128, 4, RB], BF16, tag="skipTb")
            # ============ proj1 (+ ones col) ================================
            p1e = P.tile([128, NJT, 8 * 66], BF16, tag="p1e")
            nc.vector.memset(
                p1e[:].rearrange("p j (h q) -> p j h q", q=66)[:, :, :, 64:65],
                1.0)
            for jt in range(NJT):
                ps_p = PS.tile([128, 512], F32, tag="ps")
                nc.tensor.matmul(ps_p[0:128, 0:512],
                                 xTb[:, jt * 128:(jt + 1) * 128], w1Tb[:])
                dst = p1e[:, jt, :].rearrange("p (h q) -> p h q", q=66)
                src = ps_p[0:128, 0:512].rearrange("p (h q) -> p h q", q=64)
                if jt % 2 == 0:
                    nc.scalar.activation(dst[:, :, 0:64], src, AF.Copy)
                else:
                    nc.vector.tensor_copy(dst[:, :, 0:64], src)

            # ============ layer-1 head loop =================================
            numb = P.tile([128, 4, RB], BF16, tag="numb")
            rdb = P.tile([128, 4, RB], BF16, tag="rdb")
            recbs = []
            for h in range(H1):
                numT = PSN.tile([65, 512], F32, tag="numT")
                for jt in range(NJT):
                    col = slice(jt * 16 + 8 + h, jt * 16 + 9 + h)
                    q = QF.tile([128, RB], BF16, tag="q")
                    nc.vector.tensor_scalar(q[:], gb_all[:, h, :], Wvf[:, col],
                                            Dvf[:, col], ALU.max, ALU.mult)
                    Ft = QF.tile([128, RB], BF16, tag="F")
                    dve_set = DVE_TT_JTS if h % 2 == 0 else DVE_TT_JTS[:-1]
                    eng = nc.vector if jt in dve_set else nc.gpsimd
                    eng.tensor_tensor(Ft[:], q[:], adjT[:, jt, :], ALU.mult)
                    nc.tensor.matmul(numT[0:65, 0:RB],
                                     p1e[:, jt, h * 66:h * 66 + 65], Ft[:],
                                     start=(jt == 0), stop=(jt == NJT - 1))
                den_h = WK.tile([1, RB], F32, tag="den")
                nc.scalar.activation(den_h[:], numT[64:65, 0:RB], AF.Copy)
                rec_h = WK.tile([1, RB], F32, tag="rec")
                nc.vector.reciprocal_approx_fast(rec_h[:], den_h[:])
                recb_h = P.tile([1, RB], BF16, tag=f"recb{h}")
                nc.scalar.activation(recb_h[:], rec_h[:], AF.Copy)
                recbs.append(recb_h)
                nc.scalar.activation(numb[(h % 2) * 64:(h % 2) * 64 + 64, h // 2, :],
                                     numT[0:64, 0:RB], AF.Copy)
                if h < 4:
                    ps_sk = PS.tile([128, 512], F32, tag="ps")
                    nc.tensor.matmul(ps_sk[0:128, 0:RB],
                                     ws1Tb[:, h * 128:(h + 1) * 128], xrTb[:])
                    nc.scalar.activation(skipTb[:, h, :], ps_sk[0:128, 0:RB],
                                         AF.Identity, bias=b1f[:, h:h + 1])
                if h % 2 == 1:
                    pr = h // 2
                    ps_r = PS.tile([128, 512], F32, tag="ps")
                    nc.tensor.matmul(ps_r[0:64, 0:RB], ones1b[0:1, 0:64],
                                     recbs[2 * pr][:])
                    nc.tensor.matmul(ps_r[64:128, 0:RB], ones1b[0:1, 0:64],
                                     recbs[2 * pr + 1][:])
                    nc.scalar.activation(rdb[:, pr, :], ps_r[0:128, 0:RB], AF.Copy)

            # h_out^T = elu(num/den + (skip + b1)), kept bf16, per pair
            houtb = P.tile([128, 4, RB], BF16, tag="houtb")
            for pr in range(4):
                hpre = WK.tile([128, RB], BF16, tag="hpre")
                nc.vector.tensor_mul(hpre[:], numb[:, pr, :], rdb[:, pr, :])
                u = WK.tile([128, RB], BF16, tag="u")
                nc.gpsimd.tensor_add(u[:], hpre[:], skipTb[:, pr, :])
                m0 = WK.tile([128, RB], BF16, tag="m0")
                nc.vector.tensor_scalar(m0[:], u[:], 0.0, None, ALU.min)
                e = WK.tile([128, RB], BF16, tag="e")
                nc.scalar.activation(e[:], m0[:], AF.Exp)
                em1 = WK.tile([128, RB], BF16, tag="em1")
                nc.vector.tensor_scalar(em1[:], e[:], -1.0, None, ALU.add)
                nc.vector.tensor_tensor(houtb[:, pr, :], em1[:], u[:], ALU.max)

            # ============ layer-2 local pieces ==============================
            ps_w2 = PS.tile([128, 512], F32, tag="ps")
            for kt in range(4):
                nc.tensor.matmul(ps_w2[0:128, kt * 2:kt * 2 + 2],
                                 w2n[:, kt * 128:(kt + 1) * 128], a2p[:],
                                 start=True, stop=True)
            w2tb = P.tile([128, 8], BF16, tag="w2tb")
            nc.vector.tensor_copy(w2tb[:], ps_w2[0:128, 0:8])

            # s_src2 row -> g2 broadcast (roundtrip DMA)
            ps_s2 = PS.tile([128, 512], F32, tag="ps")
            for kt in range(4):
                nc.tensor.matmul(ps_s2[0:1, 0:RB], w2tb[:, kt * 2:kt * 2 + 1],
                                 houtb[:, kt, :], start=(kt == 0), stop=(kt == 3))
            g2row = P.tile([1, RB], BF16, tag="g2row")
            nc.scalar.activation(g2row[:], ps_s2[0:1, 0:RB], AF.Exp, scale=-0.8)
            ps_g2 = PS.tile([128, 512], F32, tag="ps")
            nc.tensor.matmul(ps_g2[0:128, 0:RB], ones1b[:], g2row[:])
            g2bc = P.tile([128, RB], BF16, tag="g2bc")
            nc.scalar.activation(g2bc[:], ps_g2[0:128, 0:RB], AF.Copy)

            # p2e slot 0 (own rows): [i, 64 proj2 cols | s_tgt2]
            # moving operand w2aug [128 f', 4kt, 65]
            w2aug = P.tile([128, 4, 66], BF16, tag="w2aug")
            for kt in range(4):
                nc.vector.tensor_copy(w2aug[:, kt, 0:64], w2Tb[:, kt, :])
                nc.vector.tensor_copy(w2aug[:, kt, 64:65],
                                      w2tb[:, kt * 2 + 1:kt * 2 + 2])
            p2own = P.tile([128, 4, F1 + 1], BF16, tag="p2own")
            p2e = P.tile([128, NJT, F1 + 1], BF16, tag="p2e")
            ps_p2 = PSN.tile([128, 512], F32, tag="numT")
            for s in range(4):
                for kt in range(4):
                    nc.tensor.matmul(
                        ps_p2[0:128, s * 65:s * 65 + 65],
                        houtb[:, kt, s * 128:(s + 1) * 128],
                        w2aug[:, kt, 0:65], start=(kt == 0), stop=(kt == 3))
            nc.scalar.activation(
                p2own[:].rearrange("p s f -> p (s f)"),
                ps_p2[0:128, 0:260], AF.Copy)

            # ============ AllGather within batch group ======================
            gin = DR.tile([4, 128, 65], BF16)
            nc.sync.dma_start(gin[:].rearrange("s p f -> p s f"), p2own[:])
            gout = DR.tile([4, 4, 128, 65], BF16)
            nc.gpsimd.collective_compute(
                "AllGather", ALU.bypass, replica_groups=GROUPS,
                ins=[gin.opt()], outs=[gout.opt()])
            for q in range(2):
                nc.sync.dma_start(
                    p2e[:, 8 * q:8 * q + 8, :],
                    gout[2 * q:2 * q + 2].rearrange("c s p f -> p (c s) f"))

            # ============ layer-2 attention =================================
            st2 = P.tile([128, NJT], F32, tag="st2")
            nc.scalar.activation(st2[:], p2e[:, :, 64], AF.Copy)
            D2v = P.tile([128, NJT], F32, tag="D2v")
            nc.scalar.activation(D2v[:], st2[:], AF.Exp, scale=0.2)
            W2vf = P.tile([128, NJT], F32, tag="W2vf")
            nc.scalar.activation(W2vf[:], st2[:], AF.Exp, scale=0.8)
            # denominator column must be 1 for the ones-trick
            nc.vector.memset(p2e[:, :, F1:F1 + 1], 1.0)

            numT2 = PSN.tile([65, 512], F32, tag="numT")
            for jt in range(NJT):
                q2 = QF.tile([128, RB], BF16, tag="q")
                nc.vector.tensor_scalar(q2[:], g2bc[:], W2vf[:, jt:jt + 1],
                                        D2v[:, jt:jt + 1], ALU.max, ALU.mult)
                F2 = QF.tile([128, RB], BF16, tag="F")
                eng = nc.vector if jt in DVE_TT_JTS2 else nc.gpsimd
                eng.tensor_tensor(F2[:], q2[:], adjT[:, jt, :], ALU.mult)
                nc.tensor.matmul(numT2[0:65, 0:RB], p2e[:, jt, :],
                                 F2[:], start=(jt == 0), stop=(jt == NJT - 1))

            den2 = WK.tile([1, RB], F32, tag="den")
            nc.scalar.activation(den2[:], numT2[64:65, 0:RB], AF.Copy)
            rec2f = WK.tile([1, RB], F32, tag="rec")
            nc.vector.reciprocal_approx_fast(rec2f[:], den2[:])
            rec2b = P.tile([1, RB], BF16, tag="rec2b")
            nc.vector.tensor_copy(rec2b[:], rec2f[:])
            ps_r2 = PS.tile([128, 512], F32, tag="ps")
            nc.tensor.matmul(ps_r2[0:64, 0:RB], ones1b[0:1, 0:64], rec2b[:])
            rdb2 = P.tile([F1, RB], BF16, tag="rdb2")
            nc.scalar.activation(rdb2[:], ps_r2[0:64, 0:RB], AF.Copy)

            ps_sk2 = PS.tile([128, 512], F32, tag="ps")
            for kt in range(4):
                nc.tensor.matmul(ps_sk2[0:64, 0:RB], ws2Tb[:, kt, :],
                                 houtb[:, kt, :], start=(kt == 0), stop=(kt == 3))

            t2 = WK.tile([F1, RB], F32, tag="t2")
            nc.vector.tensor_mul(t2[:], numT2[0:64, 0:RB], rdb2[:])
            o2 = WK.tile([F1, RB], F32, tag="o2")
            nc.vector.scalar_tensor_tensor(
                o2[:], t2[:], b2f[:], ps_sk2[0:64, 0:RB], ALU.add, ALU.add)
            nc.sync.dma_start(d_out[:, :], o2[:])

    nc.compile()
    return nc


_NC_CACHE = None


def _get_nc():
    global _NC_CACHE
    if _NC_CACHE is None:
        _NC_CACHE = build_nc()
    return _NC_CACHE


def make_in_map(inp, c):
    """Host-side shard prep for core c: transpose/permute views of inputs."""
    x = np.asarray(inp["x"], np.float32)
    adj = np.asarray(inp["adj"], np.float32)
    b, r = c // 4, c % 4
    A = adj[b][r * RB:(r + 1) * RB, :].T
    return {
        "xT": np.ascontiguousarray(x[b].T),
        "xrT": np.ascontiguousarray(x[b][r * RB:(r + 1) * RB].T),
        "adjT": np.ascontiguousarray(A).astype(ml_dtypes.bfloat16),
        "w1T": np.ascontiguousarray(np.asarray(inp["W1"], np.float32).T),
        "w1": np.asarray(inp["W1"], np.float32),
        "ws1T": np.ascontiguousarray(np.asarray(inp["Wskip1"], np.float32).T),
        "asrc1": np.asarray(inp["a_src1"], np.float32),
        "atgt1": np.asarray(inp["a_tgt1"], np.float32),
        "b1": np.asarray(inp["b1"], np.float32),
        "w2": np.asarray(inp["W2"], np.float32),
        "w2T": np.ascontiguousarray(np.asarray(inp["W2"], np.float32).T),
        "ws2T": np.ascontiguousarray(np.asarray(inp["Wskip2"], np.float32).T),
        "asrc2": np.asarray(inp["a_src2"], np.float32),
        "atgt2": np.asarray(inp["a_tgt2"], np.float32),
        "b2": np.asarray(inp["b2"], np.float32),
    }


def kernel(x, adj, W1, a_src1, a_tgt1, Wskip1, b1, W2, a_src2, a_tgt2,
           Wskip2, b2):
    inp = {"x": x, "adj": adj, "W1": W1, "a_src1": a_src1, "a_tgt1": a_tgt1,
           "Wskip1": Wskip1, "b1": b1, "W2": W2, "a_src2": a_src2,
           "a_tgt2": a_tgt2, "Wskip2": Wskip2, "b2": b2}
    nc = _get_nc()
    in_maps = [make_in_map(inp, c) for c in range(NCORES)]
    res = run_bass_kernel_spmd(nc, in_maps, core_ids=list(range(NCORES)))
    out = np.empty((BS, N, F1), np.float32)
    for c in range(NCORES):
        b, r = c // 4, c % 4
        out[b, r * RB:(r + 1) * RB, :] = res.results[c]["outT"].T
    return out


# revision 33
# speedup vs baseline: 1.0052x; 1.0011x over previous
"""GAT (2-layer graph attention) Trainium2 Bass kernel, 8-core SPMD.

Sharding: data-parallel over batch (2) x row-blocks (4) -> 8 cores.
Core c handles batch b=c//4, output rows R=[512*(c%4), 512*(c%4+1)).

Key algebra: with z = s_src[i]+s_tgt[j], the GAT edge weight
exp(leaky_relu(z, 0.2)) = max(exp(z), exp(0.2 z)).  Softmax rows are
invariant to a per-row scale, so dividing row i by exp(s_src[i]) gives
unnormalized weights F[j,i] = adjT[j,i] * D[j] * max(W[j], g[i]) with
  W[j] = exp(0.8 s_tgt[j]),  D[j] = exp(0.2 s_tgt[j]),  g[i] = exp(-0.8 s_src[i])
-- no per-element transcendentals.  Per (head, j-tile) the n x n work is
one 4x-mode tensor_scalar (DVE) + one 2x tensor_tensor mask-multiply
(split DVE/Pool), feeding TensorE numerator matmuls in bf16 with a
ones-column so the softmax denominator falls out as matmul row 64.

Host-side shard prep passes x^T / adj^T / W^T slices so every transpose
is a plain strided DMA load (no PE transposes; adjacency is cast to
bf16 host-side -- {0,1} is exact).  The layer-1 -> layer-2 row exchange
is an AllGather of each core's [i, 65] proj2|s_tgt2 block within its
batch group of 4 cores.
"""

import os
import sys

for _p in ("/opt/trn_rl_repo", "/root/.axon_site/_ro/trn_rl_repo"):
    if os.path.isdir(_p) and _p not in sys.path:
        sys.path.insert(0, _p)

import ml_dtypes
import numpy as np

import concourse.bass as bass
import concourse.bacc as bacc
import concourse.mybir as mybir
from concourse import tile
from concourse.bass_utils import run_bass_kernel_spmd

F32 = mybir.dt.float32
BF16 = mybir.dt.bfloat16
AF = mybir.ActivationFunctionType
ALU = mybir.AluOpType

BS, N, FIN = 2, 2048, 128
H1, F1 = 8, 64
RB = 512          # row block per core
NJT = N // 128    # 16 j-tiles
NCORES = 8
# j-tiles handled by DVE for the mask-multiply (rest go to gpsimd/Pool)
DVE_TT_JTS = (0, 3, 6, 9, 12)
DVE_TT_JTS2 = (0, 3, 6, 9, 12)


def build_nc():
    nc = bacc.Bacc("TRN2", target_bir_lowering=False, debug=False,
                   num_devices=NCORES)

    # ---- per-core DRAM I/O (host passes transposed/permuted shards) ----
    d_xT = nc.declare_dram_parameter("xT", [FIN, N], F32, isOutput=False)
    d_xrT = nc.declare_dram_parameter("xrT", [FIN, RB], F32, isOutput=False)
    d_adjT = nc.declare_dram_parameter("adjT", [N, RB], BF16, isOutput=False)
    d_w1T = nc.declare_dram_parameter("w1T", [FIN, H1 * F1], F32, isOutput=False)
    d_w1 = nc.declare_dram_parameter("w1", [H1 * F1, FIN], F32, isOutput=False)
    d_ws1T = nc.declare_dram_parameter("ws1T", [FIN, H1 * F1], F32, isOutput=False)
    d_as1 = nc.declare_dram_parameter("asrc1", [H1, F1], F32, isOutput=False)
    d_at1 = nc.declare_dram_parameter("atgt1", [H1, F1], F32, isOutput=False)
    d_b1 = nc.declare_dram_parameter("b1", [H1 * F1], F32, isOutput=False)
    d_w2 = nc.declare_dram_parameter("w2", [F1, H1 * F1], F32, isOutput=False)
    d_w2T = nc.declare_dram_parameter("w2T", [H1 * F1, F1], F32, isOutput=False)
    d_ws2T = nc.declare_dram_parameter("ws2T", [H1 * F1, F1], F32, isOutput=False)
    d_as2 = nc.declare_dram_parameter("asrc2", [1, F1], F32, isOutput=False)
    d_at2 = nc.declare_dram_parameter("atgt2", [1, F1], F32, isOutput=False)
    d_b2 = nc.declare_dram_parameter("b2", [F1], F32, isOutput=False)
    # output: transposed row-block out^T [64, 512] (host transposes back)
    d_out = nc.declare_dram_parameter("outT", [F1, RB], F32, isOutput=True)

    GROUPS = [[0, 1, 2, 3], [4, 5, 6, 7]]

    with tile.TileContext(nc) as tc:
        with (
            tc.tile_pool(name="persist", bufs=1) as P,
            tc.tile_pool(name="work", bufs=4) as WK,
            tc.tile_pool(name="qf", bufs=14) as QF,
            tc.tile_pool(name="ps", bufs=3, space="PSUM") as PS,
            tc.tile_pool(name="psnum", bufs=4, space="PSUM") as PSN,
            tc.tile_pool(name="dram", bufs=1, space="DRAM") as DR,
        ):
            # ============ loads (all plain strided DMAs) ====================
            # small weights first (scalar queue), then x chunks + adjT (sync)
            w1n = P.tile([128, 4, FIN], F32, tag="w1n")
            nc.scalar.dma_start(w1n[:], d_w1.rearrange("(k p) c -> p k c", p=128))
            a1sT = P.tile([128, H1], F32, tag="a1sT")
            nc.scalar.dma_start(a1sT[0:F1, :], d_as1.rearrange("h f -> f h"))
            nc.scalar.dma_start(a1sT[F1:2 * F1, :], d_as1.rearrange("h f -> f h"))
            a1tT = P.tile([128, H1], F32, tag="a1tT")
            nc.scalar.dma_start(a1tT[0:F1, :], d_at1.rearrange("h f -> f h"))
            nc.scalar.dma_start(a1tT[F1:2 * F1, :], d_at1.rearrange("h f -> f h"))

            xrTf = P.tile([128, RB], F32, tag="xrTf")
            nc.sync.dma_start(xrTf[:], d_xrT[:, :])
            w1Tf = P.tile([128, H1 * F1], F32, tag="w1Tf")
            nc.sync.dma_start(w1Tf[:], d_w1T[:, :])
            xT = P.tile([128, N], F32, tag="xT")
            adjT = P.tile([128, NJT, RB], BF16, tag="adjT")
            adjq = d_adjT.rearrange("(q t p) i -> q p t i", p=128, t=4)
            nc.sync.dma_start(xT[:, 0:512], d_xT[:, 0:512])
            nc.sync.dma_start(adjT[:, 0:4, :], adjq[0])
            ws1Tf = P.tile([128, H1 * F1], F32, tag="ws1Tf")
            nc.sync.dma_start(ws1Tf[:], d_ws1T[:, :])
            b1f = P.tile([128, 4], F32, tag="b1f")
            nc.sync.dma_start(b1f[:], d_b1.rearrange("(k p) -> p k", p=128))
            for q in range(1, 4):
                nc.sync.dma_start(xT[:, q * 512:(q + 1) * 512],
                                  d_xT[:, q * 512:(q + 1) * 512])
                nc.sync.dma_start(adjT[:, 4 * q:4 * q + 4, :], adjq[q])
            w2n = P.tile([F1, H1 * F1], F32, tag="w2n")
            nc.sync.dma_start(w2n[:], d_w2[:, :])
            w2Tf = P.tile([128, 4, F1], F32, tag="w2Tf")
            nc.sync.dma_start(w2Tf[:], d_w2T.rearrange("(k p) f -> p k f", p=128))
            ws2Tf = P.tile([128, 4, F1], F32, tag="ws2Tf")
            nc.sync.dma_start(ws2Tf[:], d_ws2T.rearrange("(k p) f -> p k f", p=128))
            a2p = P.tile([F1, 2], F32, tag="a2p")
            nc.scalar.dma_start(a2p[:, 0:1], d_as2.rearrange("o f -> f o"))
            nc.scalar.dma_start(a2p[:, 1:2], d_at2.rearrange("o f -> f o"))
            b2f = P.tile([F1, 1], F32, tag="b2f")
            nc.sync.dma_start(b2f[:], d_b2.ap().rearrange("(f o) -> f o", o=1))

            # bf16 casts (DVE)
            xrTb = P.tile([128, RB], BF16, tag="xrTb")
            nc.vector.tensor_copy(xrTb[:], xrTf[:])
            xTb = P.tile([128, N], BF16, tag="xTb")
            for q in range(4):
                nc.vector.tensor_copy(xTb[:, q * 512:(q + 1) * 512],
                                      xT[:, q * 512:(q + 1) * 512])
            w1Tb = P.tile([128, H1 * F1], BF16, tag="w1Tb")
            nc.vector.tensor_copy(w1Tb[:], w1Tf[:])
            ws1Tb = P.tile([128, H1 * F1], BF16, tag="ws1Tb")
            nc.vector.tensor_copy(ws1Tb[:], ws1Tf[:])
            w2Tb = P.tile([128, 4, F1], BF16, tag="w2Tb")
            nc.vector.tensor_copy(w2Tb[:], w2Tf[:])
            ws2Tb = P.tile([128, 4, F1], BF16, tag="ws2Tb")
            nc.vector.tensor_copy(ws2Tb[:], ws2Tf[:])
            b2sb = P.tile([F1, 1], BF16, tag="b2sb")
            nc.vector.tensor_copy(b2sb[:], b2f[:])
            ones1b = P.tile([1, 128], BF16, tag="ones1b")
            nc.vector.memset(ones1b[:], 1.0)

            # ============ small exact fp32 matmuls ==========================
            # w1tilde [c=128, 16]: col h = W1_h^T a_src1[h], col 8+h tgt
            ps_wt = PS.tile([128, 512], F32, tag="ps")
            for h in range(H1):
                kt, pr = (h * F1) // 128, (h * F1) % 128
                w1slc = w1n[pr:pr + F1, kt, :]
                nc.tensor.matmul(ps_wt[0:128, h:h + 1], w1slc,
                                 a1sT[pr:pr + F1, h:h + 1])
                nc.tensor.matmul(ps_wt[0:128, 8 + h:9 + h], w1slc,
                                 a1tT[pr:pr + F1, h:h + 1])
            w1t = P.tile([128, 16], F32, tag="w1t")
            nc.vector.tensor_copy(w1t[:], ps_wt[0:128, 0:16])

            # s_src rows for our block (block 0 = own rows) -> g1b bf16
            ps_s1r = PS.tile([128, 512], F32, tag="ps")
            nc.tensor.matmul(ps_s1r[0:16, 0:RB], w1t[:], xrTf[:])
            g1b = P.tile([16, RB], BF16, tag="g1b")
            nc.scalar.activation(g1b[:], ps_s1r[0:16, 0:RB], AF.Exp, scale=-0.8)
            # selector tiles sel_h [16, 128] (row h ones) for g broadcasts
            ones16 = P.tile([16, 128], BF16, tag="ones16")
            nc.vector.memset(ones16[:], 1.0)
            sel = P.tile([16, H1 * 128], BF16, tag="sel")
            for h in range(H1):
                nc.gpsimd.affine_select(sel[:, h * 128:(h + 1) * 128],
                                        ones16[:], [[0, 128]], ALU.is_equal,
                                        0.0, base=-h, channel_multiplier=1)
            gb_all = P.tile([128, 8, RB], BF16, tag="gb_all")
            for h in range(H1):
                ps_g = PS.tile([128, 512], F32, tag="ps")
                nc.tensor.matmul(ps_g[0:128, 0:RB],
                                 sel[:, h * 128:(h + 1) * 128], g1b[:])
                if h % 2 == 0:
                    nc.scalar.activation(gb_all[:, h, :], ps_g[0:128, 0:RB],
                                         AF.Copy)
                else:
                    nc.vector.tensor_copy(gb_all[:, h, :], ps_g[0:128, 0:RB])

            # S1T [j(128 x 16 chunks), 16] = x @ w1tilde ; Wv/Dv = exp scales
            s1T = P.tile([128, NJT * 16], F32, tag="s1T")
            Wvf = P.tile([128, NJT * 16], F32, tag="Wvf")
            Dvf = P.tile([128, NJT * 16], F32, tag="Dvf")
            for q in range(4):
                ps_s1t = PS.tile([128, 512], F32, tag="ps")
                for jc in range(4 * q, 4 * q + 4):
                    nc.tensor.matmul(ps_s1t[0:128, (jc % 4) * 16:(jc % 4) * 16 + 16],
                                     xT[:, jc * 128:(jc + 1) * 128], w1t[:])
                sl = slice(q * 64, (q + 1) * 64)
                nc.vector.tensor_copy(s1T[:, sl], ps_s1t[0:128, 0:64])
                nc.scalar.activation(Wvf[:, sl], s1T[:, sl], AF.Exp, scale=0.8)
                nc.scalar.activation(Dvf[:, sl], s1T[:, sl], AF.Exp, scale=0.2)

            skipTb = P.tile([128, 4, RB], BF16, tag="skipTb")
            # ============ proj1 (+ ones col) ================================
            p1e = P.tile([128, NJT, 8 * 66], BF16, tag="p1e")
            nc.vector.memset(
                p1e[:].rearrange("p j (h q) -> p j h q", q=66)[:, :, :, 64:65],
                1.0)
            for jt in range(NJT):
                ps_p = PS.tile([128, 512], F32, tag="ps")
                nc.tensor.matmul(ps_p[0:128, 0:512],
                                 xTb[:, jt * 128:(jt + 1) * 128], w1Tb[:])
                dst = p1e[:, jt, :].rearrange("p (h q) -> p h q", q=66)
                src = ps_p[0:128, 0:512].rearrange("p (h q) -> p h q", q=64)
                if jt % 2 == 0:
                    nc.scalar.activation(dst[:, :, 0:64], src, AF.Copy)
                else:
                    nc.vector.tensor_copy(dst[:, :, 0:64], src)

            # ============ layer-1 head loop =================================
            numb = P.tile([128, 4, RB], BF16, tag="numb")
            rdb = P.tile([128, 4, RB], BF16, tag="rdb")
            recbs = []
            for h in range(H1):
                numT = PSN.tile([65, 512], F32, tag="numT")
                for jt in range(NJT):
                    col = slice(jt * 16 + 8 + h, jt * 16 + 9 + h)
                    q = QF.tile([128, RB], BF16, tag="q")
                    nc.vector.tensor_scalar(q[:], gb_all[:, h, :], Wvf[:, col],
                                            Dvf[:, col], ALU.max, ALU.mult)
                    Ft = QF.tile([128, RB], BF16, tag="F")
                    dve_set = DVE_TT_JTS if h % 2 == 0 else DVE_TT_JTS[:-1]
                    eng = nc.vector if jt in dve_set else nc.gpsimd
                    eng.tensor_tensor(Ft[:], q[:], adjT[:, jt, :], ALU.mult)
                    nc.tensor.matmul(numT[0:65, 0:RB],
                                     p1e[:, jt, h * 66:h * 66 + 65], Ft[:],
                                     start=(jt == 0), stop=(jt == NJT - 1))
                den_h = WK.tile([1, RB], F32, tag="den")
                nc.scalar.activation(den_h[:], numT[64:65, 0:RB], AF.Copy)
                rec_h = WK.tile([1, RB], F32, tag="rec")
                nc.vector.reciprocal_approx_fast(rec_h[:], den_h[:])
                recb_h = P.tile([1, RB], BF16, tag=f"recb{h}")
                nc.scalar.activation(recb_h[:], rec_h[:], AF.Copy)
                recbs.append(recb_h)
                nc.scalar.activation(numb[(h % 2) * 64:(h % 2) * 64 + 64, h // 2, :],
                                     numT[0:64, 0:RB], AF.Copy)
                if h < 4:
                    ps_sk = PS.tile([128, 512], F32, tag="ps")
                    nc.tensor.matmul(ps_sk[0:128, 0:RB],
                                     ws1Tb[:, h * 128:(h + 1) * 128], xrTb[:])
                    nc.scalar.activation(skipTb[:, h, :], ps_sk[0:128, 0:RB],
                                         AF.Identity, bias=b1f[:, h:h + 1])
                if h % 2 == 1:
                    pr = h // 2
                    ps_r = PS.tile([128, 512], F32, tag="ps")
                    nc.tensor.matmul(ps_r[0:64, 0:RB], ones1b[0:1, 0:64],
                                     recbs[2 * pr][:])
                    nc.tensor.matmul(ps_r[64:128, 0:RB], ones1b[0:1, 0:64],
                                     recbs[2 * pr + 1][:])
                    nc.scalar.activation(rdb[:, pr, :], ps_r[0:128, 0:RB], AF.Copy)

            # h_out^T = elu(num/den + (skip + b1)), kept bf16, per pair
            houtb = P.tile([128, 4, RB], BF16, tag="houtb")
            for pr in range(4):
                hpre = WK.tile([128, RB], BF16, tag="hpre")
                nc.vector.tensor_mul(hpre[:], numb[:, pr, :], rdb[:, pr, :])
                u = WK.tile([128, RB], BF16, tag="u")
                nc.gpsimd.tensor_add(u[:], hpre[:], skipTb[:, pr, :])
                m0 = WK.tile([128, RB], BF16, tag="m0")
                nc.vector.tensor_scalar(m0[:], u[:], 0.0, None, ALU.min)
                e = WK.tile([128, RB], BF16, tag="e")
                nc.scalar.activation(e[:], m0[:], AF.Exp)
                em1 = WK.tile([128, RB], BF16, tag="em1")
                nc.vector.tensor_scalar(em1[:], e[:], -1.0, None, ALU.add)
                nc.vector.tensor_tensor(houtb[:, pr, :], em1[:], u[:], ALU.max)

            # ============ layer-2 local pieces ==============================
            ps_w2 = PS.tile([128, 512], F32, tag="ps")
            for kt in range(4):
                nc.tensor.matmul(ps_w2[0:128, kt * 2:kt * 2 + 2],
                                 w2n[:, kt * 128:(kt + 1) * 128], a2p[:],
                                 start=True, stop=True)
            w2tb = P.tile([128, 8], BF16, tag="w2tb")
            nc.vector.tensor_copy(w2tb[:], ps_w2[0:128, 0:8])

            # s_src2 row -> g2 broadcast (roundtrip DMA)
            ps_s2 = PS.tile([128, 512], F32, tag="ps")
            for kt in range(4):
                nc.tensor.matmul(ps_s2[0:1, 0:RB], w2tb[:, kt * 2:kt * 2 + 1],
                                 houtb[:, kt, :], start=(kt == 0), stop=(kt == 3))
            g2row = P.tile([1, RB], BF16, tag="g2row")
            nc.scalar.activation(g2row[:], ps_s2[0:1, 0:RB], AF.Exp, scale=-0.8)
            ps_g2 = PS.tile([128, 512], F32, tag="ps")
            nc.tensor.matmul(ps_g2[0:128, 0:RB], ones1b[:], g2row[:])
            g2bc = P.tile([128, RB], BF16, tag="g2bc")
            nc.scalar.activation(g2bc[:], ps_g2[0:128, 0:RB], AF.Copy)

            # p2e slot 0 (own rows): [i, 64 proj2 cols | s_tgt2]
            # moving operand w2aug [128 f', 4kt, 65]
            w2aug = P.tile([128, 4, 66], BF16, tag="w2aug")
            for kt in range(4):
                nc.vector.tensor_copy(w2aug[:, kt, 0:64], w2Tb[:, kt, :])
                nc.vector.tensor_copy(w2aug[:, kt, 64:65],
                                      w2tb[:, kt * 2 + 1:kt * 2 + 2])
            p2own = P.tile([128, 4, F1 + 1], BF16, tag="p2own")
            p2e = P.tile([128, NJT, F1 + 1], BF16, tag="p2e")
            ps_p2 = PSN.tile([128, 512], F32, tag="numT")
            for s in range(4):
                for kt in range(4):
                    nc.tensor.matmul(
                        ps_p2[0:128, s * 65:s * 65 + 65],
                        houtb[:, kt, s * 128:(s + 1) * 128],
                        w2aug[:, kt, 0:65], start=(kt == 0), stop=(kt == 3))
            nc.scalar.activation(
                p2own[:].rearrange("p s f -> p (s f)"),
                ps_p2[0:128, 0:260], AF.Copy)

            # ============ AllGather within batch group ======================
            gin = DR.tile([4, 128, 65], BF16)
            nc.sync.dma_start(gin[:].rearrange("s p f -> p s f"), p2own[:])
            gout = DR.tile([4, 4, 128, 65], BF16)
            nc.gpsimd.collective_compute(
                "AllGather", ALU.bypass, replica_groups=GROUPS,
                ins=[gin.opt()], outs=[gout.opt()])
            for q in range(2):
                nc.sync.dma_start(
                    p2e[:, 8 * q:8 * q + 8, :],
                    gout[2 * q:2 * q + 2].rearrange("c s p f -> p (c s) f"))

            # ============ layer-2 attention =================================
            D2v = P.tile([128, NJT], F32, tag="D2v")
            nc.scalar.activation(D2v[:], p2e[:, :, 64], AF.Exp, scale=0.2)
            W2vf = P.tile([128, NJT], F32, tag="W2vf")
            nc.scalar.activation(W2vf[:], p2e[:, :, 64], AF.Exp, scale=0.8)
            # denominator column must be 1 for the ones-trick
            nc.vector.memset(p2e[:, :, F1:F1 + 1], 1.0)

            numT2 = PSN.tile([65, 512], F32, tag="numT")
            for jt in range(NJT):
                q2 = QF.tile([128, RB], BF16, tag="q")
                nc.vector.tensor_scalar(q2[:], g2bc[:], W2vf[:, jt:jt + 1],
                                        D2v[:, jt:jt + 1], ALU.max, ALU.mult)
                F2 = QF.tile([128, RB], BF16, tag="F")
                eng = nc.vector if jt in DVE_TT_JTS2 else nc.gpsimd
                eng.tensor_tensor(F2[:], q2[:], adjT[:, jt, :], ALU.mult)
                nc.tensor.matmul(numT2[0:65, 0:RB], p2e[:, jt, :],
                                 F2[:], start=(jt == 0), stop=(jt == NJT - 1))

            den2 = WK.tile([1, RB], F32, tag="den")
            nc.scalar.activation(den2[:], numT2[64:65, 0:RB], AF.Copy)
            rec2f = WK.tile([1, RB], F32, tag="rec")
            nc.vector.reciprocal_approx_fast(rec2f[:], den2[:])
            rec2b = P.tile([1, RB], BF16, tag="rec2b")
            nc.vector.tensor_copy(rec2b[:], rec2f[:])
            ps_r2 = PS.tile([128, 512], F32, tag="ps")
            nc.tensor.matmul(ps_r2[0:64, 0:RB], ones1b[0:1, 0:64], rec2b[:])
            rdb2 = P.tile([F1, RB], BF16, tag="rdb2")
            nc.scalar.activation(rdb2[:], ps_r2[0:64, 0:RB], AF.Copy)

            ps_sk2 = PS.tile([128, 512], F32, tag="ps")
            for kt in range(4):
                nc.tensor.matmul(ps_sk2[0:64, 0:RB], ws2Tb[:, kt, :],
                                 houtb[:, kt, :], start=(kt == 0), stop=(kt == 3))

            t2 = WK.tile([F1, RB], F32, tag="t2")
            nc.vector.tensor_mul(t2[:], numT2[0:64, 0:RB], rdb2[:])
            o2 = WK.tile([F1, RB], F32, tag="o2")
            nc.vector.scalar_tensor_tensor(
                o2[:], t2[:], b2f[:], ps_sk2[0:64, 0:RB], ALU.add, ALU.add)
            nc.sync.dma_start(d_out[:, :], o2[:])

    nc.compile()
    return nc


_NC_CACHE = None


def _get_nc():
    global _NC_CACHE
    if _NC_CACHE is None:
        _NC_CACHE = build_nc()
    return _NC_CACHE


def make_in_map(inp, c):
    """Host-side shard prep for core c: transpose/permute views of inputs."""
    x = np.asarray(inp["x"], np.float32)
    adj = np.asarray(inp["adj"], np.float32)
    b, r = c // 4, c % 4
    A = adj[b][r * RB:(r + 1) * RB, :].T
    return {
        "xT": np.ascontiguousarray(x[b].T),
        "xrT": np.ascontiguousarray(x[b][r * RB:(r + 1) * RB].T),
        "adjT": np.ascontiguousarray(A).astype(ml_dtypes.bfloat16),
        "w1T": np.ascontiguousarray(np.asarray(inp["W1"], np.float32).T),
        "w1": np.asarray(inp["W1"], np.float32),
        "ws1T": np.ascontiguousarray(np.asarray(inp["Wskip1"], np.float32).T),
        "asrc1": np.asarray(inp["a_src1"], np.float32),
        "atgt1": np.asarray(inp["a_tgt1"], np.float32),
        "b1": np.asarray(inp["b1"], np.float32),
        "w2": np.asarray(inp["W2"], np.float32),
        "w2T": np.ascontiguousarray(np.asarray(inp["W2"], np.float32).T),
        "ws2T": np.ascontiguousarray(np.asarray(inp["Wskip2"], np.float32).T),
        "asrc2": np.asarray(inp["a_src2"], np.float32),
        "atgt2": np.asarray(inp["a_tgt2"], np.float32),
        "b2": np.asarray(inp["b2"], np.float32),
    }


def kernel(x, adj, W1, a_src1, a_tgt1, Wskip1, b1, W2, a_src2, a_tgt2,
           Wskip2, b2):
    inp = {"x": x, "adj": adj, "W1": W1, "a_src1": a_src1, "a_tgt1": a_tgt1,
           "Wskip1": Wskip1, "b1": b1, "W2": W2, "a_src2": a_src2,
           "a_tgt2": a_tgt2, "Wskip2": Wskip2, "b2": b2}
    nc = _get_nc()
    in_maps = [make_in_map(inp, c) for c in range(NCORES)]
    res = run_bass_kernel_spmd(nc, in_maps, core_ids=list(range(NCORES)))
    out = np.empty((BS, N, F1), np.float32)
    for c in range(NCORES):
        b, r = c // 4, c % 4
        out[b, r * RB:(r + 1) * RB, :] = res.results[c]["outT"].T
    return out
